# revision 38
# baseline (speedup 1.0000x reference)
"""Multi-head attention Trainium2 Bass kernel.

Problem: B=4, N=M=2048, DM=512, H=8, DH=64, DO=512, fp32.
Sharding: 8 cores = (batch b, row-half) -- each core computes full attention
for 1024 query rows of one batch. No collectives.

Per-core dataflow (v3: bf16 value path, engine-balanced, JIT-pipelined):
  - K/Q: PE-transpose fp32r 128x128 blocks; kTf/qTf = W.T @ XT (fp32r);
    transpose PSUM->SBUF copies on DVE (ScalarE stays exp-only)
  - V: bf16 quantize (Pool) -> PE-transpose bf16 -> bf16 proj -> vha =
    bf16(vh + vb) with a ones column per head (v-bias folded: sum w = 1)
  - scoresT[m,n] = kh @ qhT fp32r (2 heads row-packed via tile_position)
  - exp on ScalarE -> ex bf16
  - ohT[65, n] += vha.T @ ex (bf16; row 64 = softmax denominator)
  - norm: recip(sum) broadcast via PE ones-matmul; mh = bf16(oh * bc)
  - out[n, do] = sum_h mh_h.T @ Wp_h (bf16) + pb (pre-broadcast tile)
JIT schedule: K-groups 1-3 transposed/projected inside the (nb0, hp) loops
(split by head-pair) so exp starts after a short prefix and ScalarE stays
the pacing engine. fp8 was tried and abandoned: with softmax N_eff ~ 44
and the out-proj amplifying elementwise noise ~1.3x, 3-6% fp8 quantization
lands above the 2e-2 max-rel gate.
"""
import os
import sys

sys.path.insert(0, "/opt/trn_rl_repo")

import numpy as np
import ml_dtypes

import concourse.bass as bass
import concourse.mybir as mybir
import concourse.tile as tile
from concourse import bacc
from concourse.bass_utils import run_bass_kernel_spmd

F32 = mybir.dt.float32
F32R = mybir.dt.float32r
BF16 = mybir.dt.bfloat16
EXP = mybir.ActivationFunctionType.Exp
ADD = mybir.AluOpType.add
MULT = mybir.AluOpType.mult

P = 128
DM = 512
HDH = 512
DH = 64
H = 8
NB = 1024     # query rows per core
M = 2048      # kv rows
DO = 512
N_MT = M // P
N_QT = NB // P

_CACHED = {}
LAST_EXEC_NS = None


def _build():
    nc = bacc.Bacc("TRN2", target_bir_lowering=False, debug=False)

    d_q = nc.declare_dram_parameter("q", [NB, DM], F32, isOutput=False)
    d_k = nc.declare_dram_parameter("k", [M, DM], F32, isOutput=False)
    d_v = nc.declare_dram_parameter("v", [M, DM], F32, isOutput=False)
    d_wq = nc.declare_dram_parameter("wq", [DM, HDH], F32R, isOutput=False)
    d_wk = nc.declare_dram_parameter("wk", [DM, HDH], F32R, isOutput=False)
    d_wv = nc.declare_dram_parameter("wv", [DM, HDH], BF16, isOutput=False)
    d_wp = nc.declare_dram_parameter("wp", [HDH, DO], BF16, isOutput=False)
    d_qb = nc.declare_dram_parameter("qb", [P, 4], F32, isOutput=False)
    d_kb = nc.declare_dram_parameter("kb", [P, 4], F32, isOutput=False)
    d_vb = nc.declare_dram_parameter("vb", [1, DO], F32R, isOutput=False)
    d_pb = nc.declare_dram_parameter("pb", [1, DO], F32R, isOutput=False)
    d_id = nc.declare_dram_parameter("ident", [P, P], F32, isOutput=False)
    d_id8 = nc.declare_dram_parameter("ident8", [P, P], BF16, isOutput=False)
    d_ones = nc.declare_dram_parameter("ones", [P, P], F32R, isOutput=False)
    d_out = nc.declare_dram_parameter("out", [NB, DO], F32, isOutput=True)
    dbg_on = os.environ.get("KDBG") == "1"
    d_dbg = (nc.declare_dram_parameter("dbg", [P, 8 * DO], F32, isOutput=True)
             if dbg_on else None)

    with tile.TileContext(nc) as tc:
        from contextlib import ExitStack
        with ExitStack() as ctx:
            persist = ctx.enter_context(tc.tile_pool(name="persist", bufs=1))
            aw = ctx.enter_context(tc.tile_pool(name="aw", bufs=1))
            raw = ctx.enter_context(tc.tile_pool(name="raw", bufs=4))
            vraw = ctx.enter_context(tc.tile_pool(name="vraw", bufs=3))
            vtt_pool = ctx.enter_context(tc.tile_pool(name="vtt", bufs=7))
            exp_pool = ctx.enter_context(tc.tile_pool(name="expp", bufs=6))
            nm = ctx.enter_context(tc.tile_pool(name="nm", bufs=2))
            nm1 = ctx.enter_context(tc.tile_pool(name="nm1", bufs=2))
            nm2 = ctx.enter_context(tc.tile_pool(name="nm2", bufs=1))
            ps = ctx.enter_context(tc.tile_pool(name="ps", bufs=2, space="PSUM"))
            ps_sc = ctx.enter_context(tc.tile_pool(name="ps_sc", bufs=2, space="PSUM"))
            ps_oh = ctx.enter_context(tc.tile_pool(name="ps_oh", bufs=1, space="PSUM"))

            # --- constants (critical-path first) ---
            ident = persist.tile([P, P], F32R, tag="ident", name="ident")
            nc.gpsimd.dma_start(ident[:], d_id[:].bitcast(F32R))
            ident8 = persist.tile([P, P], BF16, tag="ident8", name="ident8")
            nc.gpsimd.dma_start(ident8[:], d_id8[:])
            qb = persist.tile([P, 4], F32, tag="qb", name="qb")
            nc.gpsimd.dma_start(qb[:], d_qb[:])
            kb = persist.tile([P, 4], F32, tag="kb", name="kb")
            nc.gpsimd.dma_start(kb[:], d_kb[:])
            ones = persist.tile([P, P], F32R, tag="ones", name="ones")
            vbrow = persist.tile([1, DO], F32R, tag="vbrow", name="vbrow")
            pbrow = persist.tile([1, DO], F32R, tag="pbrow", name="pbrow")
            vb_bc = persist.tile([P, DO], F32, tag="vb_bc", name="vb_bc")
            pb_bc = persist.tile([P, DO], F32, tag="pb_bc", name="pb_bc")
            wp_sb = [persist.tile([DH, DO], BF16, tag=f"wp{h}", name=f"wp{h}")
                     for h in range(H)]

            # --- persistent activations ---
            kTf = [persist.tile([P, M], F32R, tag=f"kTf{i}", name=f"kTf{i}")
                   for i in range(4)]
            qTf = [persist.tile([P, NB], F32R, tag=f"qTf{i}", name=f"qTf{i}")
                   for i in range(4)]
            # bf16 vh (+vb) with a ones column per head: [p, mt, h*65+c]
            vha = persist.tile([P, N_MT, 520], BF16, tag="vha", name="vha")
            mh = [persist.tile([DH, NB], BF16, tag=f"mh{h}", name=f"mh{h}")
                  for h in range(H)]
            # K transpose groups 0..3 + Q ns0 persist so each hp-boundary
            # projects its own head-pair slice just in time
            kts_g = [persist.tile([P, 4, 512], F32R, tag=f"ktsg{g}",
                                  name=f"ktsg{g}") for g in range(4)]
            qts0 = persist.tile([P, 4, 512], F32R, tag="qts0", name="qts0")

            wk_sb = [aw.tile([P, HDH], F32R, tag=f"wk{dc}", name=f"wk{dc}")
                     for dc in range(4)]
            wq_sb = [aw.tile([P, HDH], F32R, tag=f"wq{dc}", name=f"wq{dc}")
                     for dc in range(4)]
            wv_sb = [aw.tile([P, HDH], BF16, tag=f"wv{dc}", name=f"wv{dc}")
                     for dc in range(4)]

            def load_w(d_w, w_sb):
                for dc in range(4):
                    nc.gpsimd.dma_start(w_sb[dc][:], d_w[dc * P:(dc + 1) * P, :])

            def transpose_tiles(d_src, t0, n_tiles, ts, pool_raw, dma_eng=None):
                """Transpose rows [t0*P, (t0+n_tiles)*P) of d_src into ts
                ([dm-chunk, dc, row]); fp32r path (K/Q)."""
                for j in range(n_tiles):
                    rn = pool_raw.tile([P, DM], F32R, tag="araw", name="araw")
                    (dma_eng or nc.sync).dma_start(
                        rn[:], d_src[(t0 + j) * P:(t0 + j + 1) * P, :].bitcast(F32R))
                    pst = ps.tile([P, DM], F32R, tag="pj", name="pj")
                    for dc in range(4):
                        nc.tensor.transpose(
                            pst[:, dc * P:(dc + 1) * P], rn[:, dc * P:(dc + 1) * P],
                            ident[:],
                        )
                    nc.vector.tensor_copy(
                        ts[:, :, j * P:(j + 1) * P],
                        pst.rearrange("p (a b) -> p a b", a=4),
                    )

            def proj_ht(w_sb, ts, out_tile, cslice, bias, bcol):
                """One [128, 512] projection tile: 4-chunk fp32r accum + bias."""
                pp = ps.tile([P, 512], F32, tag="pj", name="pj")
                for dc in range(4):
                    nc.tensor.matmul(
                        pp[:], w_sb[dc][:, bcol * P:(bcol + 1) * P], ts[:, dc, :],
                        start=(dc == 0), stop=(dc == 3),
                    )
                nc.vector.tensor_scalar(
                    out_tile[:, cslice], pp[:], bias[:, bcol:bcol + 1], None, ADD,
                )

            vtts = {}

            def emit_v_fetch(mt):
                """Load + PE-transpose one V m-tile into a vtt ring slot."""
                vn = vraw.tile([P, DM], F32R, tag="vraw", name="vraw")
                nc.sync.dma_start(vn[:],
                                  d_v[mt * P:(mt + 1) * P, :].bitcast(F32R))
                pst = ps.tile([P, DM], F32R, tag="pj", name="pj")
                for dc in range(4):
                    nc.tensor.transpose(
                        pst[:, dc * P:(dc + 1) * P], vn[:, dc * P:(dc + 1) * P],
                        ident[:],
                    )
                vtt = vtt_pool.tile([P, 4, P], BF16, tag="vtt", name="vtt")
                nc.vector.tensor_copy(vtt[:],
                                      pst.rearrange("p (a b) -> p a b", a=4))
                vtts[mt] = vtt

            def emit_v_proj(mt):
                vtt = vtts.pop(mt)
                pp = ps.tile([P, DO], F32, tag="pj", name="pj")
                for dc in range(4):
                    nc.tensor.matmul(
                        pp[:], vtt[:, dc, :], wv_sb[dc][:],
                        start=(dc == 0), stop=(dc == 3),
                    )
                nc.vector.tensor_tensor(
                    vha.rearrange("p a (h c) -> p a h c", c=65)[:, mt, :, 0:64],
                    pp.rearrange("p (h c) -> p h c", h=H),
                    vb_bc.rearrange("p (h c) -> p h c", h=H),
                    ADD,
                )

            def emit_out_group(nt):
                po = ps.tile([P, DO], F32, tag="pj", name="pj")
                for h in range(H):
                    nc.tensor.matmul(
                        po[:], mh[h][:, nt * P:(nt + 1) * P], wp_sb[h][:],
                        start=(h == 0), stop=(h == H - 1), skip_group_check=True,
                    )
                ot = nm.tile([P, DO], F32, tag="rc", name="rc")
                nc.vector.tensor_tensor(ot[:], po[:], pb_bc[:], ADD)
                nc.gpsimd.dma_start(d_out[nt * P:(nt + 1) * P, :], ot[:])

            # === Prefix: minimal head-pair-0 slice, then JIT the rest ===
            load_w(d_wk, wk_sb)
            transpose_tiles(d_k, 0, 4, kts_g[0], raw)
            proj_ht(wk_sb, kts_g[0], kTf[0], slice(0, 512), kb, 0)
            load_w(d_wq, wq_sb)
            transpose_tiles(d_q, 0, 4, qts0, raw, dma_eng=nc.scalar)
            proj_ht(wq_sb, qts0, qTf[0], slice(0, 512), qb, 0)
            load_w(d_wv, wv_sb)
            nc.gpsimd.dma_start(ones[:], d_ones[:])
            nc.gpsimd.dma_start(vbrow[:], d_vb[:])
            nc.gpsimd.dma_start(pbrow[:], d_pb[:])
            # broadcast vb/pb rows to all 128 partitions via ones-matmul
            for row, bc in ((vbrow, vb_bc), (pbrow, pb_bc)):
                bps = ps.tile([P, DO], F32, tag="pj", name="pj")
                nc.tensor.matmul(bps[:], ones[0:1, :], row[:],
                                 start=True, stop=True)
                nc.vector.tensor_copy(bc[:], bps[:])
            # ones columns of vha (col 64 of each head group)
            nc.vector.tensor_copy(
                vha.rearrange("p a (h c) -> p a h c", c=65)[:, :, :, 64:65],
                ones[:, 0:N_MT * 8].bitcast(F32).rearrange(
                    "p (a h) -> p a h", a=N_MT)[:, :, :, None],
            )
            for mt in range(6):
                emit_v_fetch(mt)
            emit_v_proj(0)
            emit_v_proj(1)

            def dbg_tap(slot, src_ap):
                if not dbg_on:
                    return
                t = nm2.tile([P, DO], F32, tag="dbgt", name="dbgt")
                nc.vector.memset(t[:], 0.0)
                nc.vector.tensor_copy(t[0:src_ap.shape[0], 0:src_ap.shape[-1]],
                                      src_ap)
                nc.gpsimd.dma_start(d_dbg[:, slot * DO:(slot + 1) * DO], t[:])

            def k_grp_dma(g, store):
                for j in range(4):
                    rn = raw.tile([P, DM], F32R, tag="araw", name="araw")
                    nc.scalar.dma_start(
                        rn[:],
                        d_k[(4 * g + j) * P:(4 * g + j + 1) * P, :].bitcast(F32R))
                    store[(g, j)] = rn

            def k_grp_transpose(g, store):
                for j in range(4):
                    rn = store.pop((g, j))
                    pst = ps.tile([P, DM], F32R, tag="pj", name="pj")
                    for dc in range(4):
                        nc.tensor.transpose(
                            pst[:, dc * P:(dc + 1) * P],
                            rn[:, dc * P:(dc + 1) * P], ident[:],
                        )
                    nc.vector.tensor_copy(
                        kts_g[g][:, :, j * P:(j + 1) * P],
                        pst.rearrange("p (a b) -> p a b", a=4),
                    )

            # === Main: attention; K/Q/V leftovers JIT-scheduled in nb0 ===
            def make_emit_av(oh, hp):
                def emit_av(mu, ex):
                    for ab in range(2):
                        h = 2 * hp + ab
                        for j in range(2):
                            mt = 2 * mu + j
                            nc.tensor.matmul(
                                oh[ab][0:65, :],
                                vha[:, mt, h * 65:h * 65 + 65],
                                ex[ab][:, j, :],
                                start=(mu == 0 and j == 0),
                                stop=(mu == N_MT // 2 - 1 and j == 1),
                            )
                return emit_av

            kraws = {}
            qraws = {}
            qts1 = None
            pending = []
            k_grp_dma(1, kraws)
            for nb in range(2):
                for hp in range(4):
                    ns = slice(nb * 512, (nb + 1) * 512)
                    oh = {}
                    for ab in range(2):
                        oh[ab] = ps_oh.tile([P, 512], F32, tag=f"oh{ab}",
                                            name=f"oh{ab}")
                    emit_av = make_emit_av(oh, hp)
                    prev_ex = None
                    for mu in range(N_MT // 2):
                        # --- attention first: JIT work must not gate scores ---
                        ex = {}
                        for ab in range(2):
                            base = ab * 64
                            sc = ps_sc.tile([P, 2, 512], F32, tag="sc", name="sc")
                            for j in range(2):
                                mt = 2 * mu + j
                                nc.tensor.matmul(
                                    sc[:, j, :],
                                    kTf[hp][base:base + 64, mt * P:(mt + 1) * P],
                                    qTf[hp][base:base + 64, ns],
                                    start=True, stop=True,
                                    tile_position=(base, 0),
                                )
                            ex[ab] = exp_pool.tile([P, 2, 512], BF16, tag="ex",
                                                   name="ex")
                            nc.scalar.activation(ex[ab][:], sc[:], EXP)
                        if mu == 0:
                            for fn in pending:
                                fn()
                            pending = []
                            if dbg_on and nb == 0 and hp == 0:
                                dbg_tap(0, kTf[0][:, 0:512].bitcast(F32))
                                dbg_tap(1, qTf[0][:, 0:512].bitcast(F32))
                                dbg_tap(2, vha[:, 0, 0:512])
                                dbg_tap(3, ex[0][:, 0, :])
                        # AV skewed one mu behind scores: PE never waits exp
                        if mu > 0:
                            emit_av(mu - 1, prev_ex)
                        prev_ex = ex
                        # --- JIT phase-A leftovers, consumed >= 1 mu later ---
                        if nb == 0:
                            if hp == 0:
                                if mu in (0, 2):
                                    # raws for group g arrive ~2 mus early
                                    k_grp_dma(mu // 2 + 2, kraws)
                                if mu in (0, 2, 4):
                                    g = mu // 2 + 1
                                    k_grp_transpose(g, kraws)
                                    proj_ht(wk_sb, kts_g[g], kTf[0],
                                            slice(g * 512, (g + 1) * 512), kb, 0)
                                if mu < 5:
                                    emit_v_fetch(2 * mu + 6)
                                    emit_v_fetch(2 * mu + 7)
                                if mu < 7:
                                    emit_v_proj(2 * mu + 2)
                                    emit_v_proj(2 * mu + 3)
                                if mu == 5:
                                    for h in range(H):
                                        nc.gpsimd.dma_start(
                                            wp_sb[h][:],
                                            d_wp[h * DH:(h + 1) * DH, :])
                            elif mu < 3:
                                g = mu + 1
                                proj_ht(wk_sb, kts_g[g], kTf[hp],
                                        slice(g * 512, (g + 1) * 512), kb, hp)
                            if hp == 2 and mu == 5:
                                for j in range(4):
                                    rn = raw.tile([P, DM], F32R, tag="araw",
                                                  name="araw")
                                    nc.sync.dma_start(
                                        rn[:],
                                        d_q[(4 + j) * P:(4 + j + 1) * P, :]
                                        .bitcast(F32R))
                                    qraws[j] = rn
                            if hp == 3:
                                # q ns1: transpose into kts_g[1] (free after
                                # this hp's grp1 projection at mu 0)
                                if mu in (0, 1, 2, 3):
                                    j = mu
                                    qts1 = kts_g[1]
                                    rn = qraws.pop(j)
                                    pst = ps.tile([P, DM], F32R, tag="pj",
                                                  name="pj")
                                    for dc in range(4):
                                        nc.tensor.transpose(
                                            pst[:, dc * P:(dc + 1) * P],
                                            rn[:, dc * P:(dc + 1) * P],
                                            ident[:],
                                        )
                                    nc.vector.tensor_copy(
                                        qts1[:, :, j * P:(j + 1) * P],
                                        pst.rearrange("p (a b) -> p a b", a=4),
                                    )
                                if mu == 3:
                                    proj_ht(wq_sb, kts_g[1], qTf[0],
                                            slice(512, 1024), qb, 0)
                            if mu == 5 and hp < 3:
                                # JIT the next head-pair's grp0/ns0 projections
                                proj_ht(wk_sb, kts_g[0], kTf[hp + 1],
                                        slice(0, 512), kb, hp + 1)
                                proj_ht(wq_sb, qts0, qTf[hp + 1],
                                        slice(0, 512), qb, hp + 1)
                        elif hp < 3 and mu == 5:
                            # q ns1 projection for the next nb1 head-pair
                            proj_ht(wq_sb, kts_g[1], qTf[hp + 1],
                                    slice(512, 1024), qb, hp + 1)
                    emit_av(N_MT // 2 - 1, prev_ex)
                    # normalization phase 1 (DVE): recips; phase 2 deferred
                    from concourse.dve_ops import (
                        RECIP_APPROX_FAST_CONSTS, RECIPROCAL_APPROX_FAST)
                    _c = RECIP_APPROX_FAST_CONSTS
                    for ab in range(2):
                        h = 2 * hp + ab
                        sums_sb = nm2.tile([P, 512], F32, tag="sums",
                                           name="sums")
                        nc.vector.tensor_copy(sums_sb[64:65, :],
                                              oh[ab][64:65, :])
                        rr = nm1.tile([P, 512], F32R, tag="rr", name="rr")
                        nc.vector._custom_dve(
                            RECIPROCAL_APPROX_FAST, out=rr[:], in0=sums_sb[:],
                            s0=_c["s0"], s1=_c["s1"], imm2=_c["imm2"],
                        )

                        def phase2(h=h, rr=rr, oh_t=oh[ab], ns=ns):
                            bc_ps = ps.tile([64, 512], F32, tag="pj", name="pj")
                            nc.tensor.matmul(
                                bc_ps[:], ones[64:65, 0:DH], rr[64:65, :],
                                start=True, stop=True,
                            )
                            bc_sb = nm2.tile([64, 512], F32, tag="bcs",
                                             name="bcs")
                            nc.vector.tensor_copy(bc_sb[:], bc_ps[:])
                            nc.vector.tensor_tensor(
                                mh[h][:, ns], oh_t[0:64, :], bc_sb[:], MULT)

                        pending.append(phase2)
                    if nb == 1:
                        # fill ScalarE-bound window with nb0's output projection
                        emit_out_group(hp)
            for fn in pending:
                fn()
            pending = []

            # === tail: second n-half output projection ===
            for nt in range(4, N_QT):
                emit_out_group(nt)

    nc.compile()
    return nc


def kernel(query, key, value, query_kernel, key_kernel, value_kernel,
           projection_kernel, q_bias, k_bias, v_bias, projection_bias):
    query = np.ascontiguousarray(np.asarray(query, dtype=np.float32))
    key = np.ascontiguousarray(np.asarray(key, dtype=np.float32))
    value = np.ascontiguousarray(np.asarray(value, dtype=np.float32))
    scale = np.float32(1.0 / 8.0)  # 1/sqrt(DH)
    bf16 = ml_dtypes.bfloat16

    wq = np.ascontiguousarray(
        (np.asarray(query_kernel, np.float32) * scale).transpose(1, 0, 2).reshape(DM, HDH))
    wk = np.ascontiguousarray(
        np.asarray(key_kernel, np.float32).transpose(1, 0, 2).reshape(DM, HDH))
    wv = np.ascontiguousarray(
        np.asarray(value_kernel, np.float32).transpose(1, 0, 2).reshape(DM, HDH)
    ).astype(bf16)
    wp = np.ascontiguousarray(
        np.asarray(projection_kernel, np.float32).reshape(HDH, DO)).astype(bf16)
    qb = np.ascontiguousarray(
        (np.asarray(q_bias, np.float32) * scale).reshape(HDH).reshape(4, P).T)
    kb = np.ascontiguousarray(np.asarray(k_bias, np.float32).reshape(HDH).reshape(4, P).T)
    vb = np.ascontiguousarray(np.asarray(v_bias, np.float32).reshape(1, HDH))
    pb = np.ascontiguousarray(np.asarray(projection_bias, np.float32).reshape(1, DO))
    ident = np.eye(P, dtype=np.float32)
    ident8 = np.eye(P, dtype=np.float32).astype(bf16)
    ones = np.ones((P, P), dtype=np.float32)

    if "nc" not in _CACHED:
        _CACHED["nc"] = _build()
    nc = _CACHED["nc"]

    shared = dict(wq=wq, wk=wk, wv=wv, wp=wp, qb=qb, kb=kb, vb=vb, pb=pb,
                  ident=ident, ident8=ident8, ones=ones)
    in_maps = []
    for c in range(8):
        b, half = c // 2, c % 2
        in_maps.append(dict(
            q=np.ascontiguousarray(query[b, half * NB:(half + 1) * NB, :]),
            k=key[b], v=value[b], **shared))

    trace = os.environ.get("KERNEL_TRACE", "0") == "1"
    try:
        res = run_bass_kernel_spmd(nc, in_maps, core_ids=list(range(8)), trace=trace)
    except ModuleNotFoundError:
        # axon NTFF profiling hook unavailable -- run without tracing
        res = run_bass_kernel_spmd(nc, in_maps, core_ids=list(range(8)), trace=False)
    global LAST_EXEC_NS
    LAST_EXEC_NS = res.exec_time_ns
    if trace and res.exec_time_ns is not None:
        print(f"HW exec time: {res.exec_time_ns} ns")
        if res.instructions_and_trace is not None:
            print(f"trace: {res.instructions_and_trace[1]}")

    B = query.shape[0]
    out = np.empty((B, 2 * NB, DO), dtype=np.float32)
    for c in range(8):
        b, half = c // 2, c % 2
        out[b, half * NB:(half + 1) * NB, :] = res.results[c]["out"]
    return out
